# revision 17
# baseline (speedup 1.0000x reference)
"""Trainium2 8-core Bass kernel for nn_Attention_7112465842253.

Token-sharded attention: 512 tokens/core (cores 0-3 = batch 0, 4-7 = batch 1).
Per core: QKV projection in bf16 (q/k in transposed [chan, tok] layout, v in
natural [tok, chan]), RMSNorm via weighted-sumsq matmul; the k-side rstd is
NOT applied to k at all -- it rides the AllGather as a 512-element row and is
folded into the softmax exp as a per-partition (per-k-token) activation
scale. RoPE as x*C + shuffle(x)*S with a DVE stream_shuffle partition swap.
Two AllGathers (k first, then v+rstd_k) inside each 4-core batch group,
non-causal attention in scoresT layout (k-tokens on partitions; softmax
denominator from a ones column appended to V; Exp fused into the PSUM->SBUF
eviction on ScalarE with the rstd_k scale), normalization via DVE reciprocal
+ gpsimd partition_broadcast (no DRAM round trips), then the output
projection. Host does layout prep and reassembly.
"""

import numpy as np

B, N, DIN, DIM, H, HD = 2, 2048, 1024, 1024, 16, 64
NCORE = 8
TOK = 512
EPS = 1e-6
BASE = 10000.0
KC = N // 128        # 16 k-token chunks

_CACHE = {}

# stream_shuffle mask: per-partition permutation applied within each
# 32-partition block; swaps 16-halves (rope x1/x2 blocks) within 32-blocks
SWAP_MASK = list(range(16, 32)) + list(range(0, 16))


def _build_nc(dbg=None, single=False, zero_bias=False):
    import concourse.bass as bass
    import concourse.tile as tile
    from concourse import bacc, mybir
    from contextlib import ExitStack

    BF = mybir.dt.bfloat16
    F32 = mybir.dt.float32
    AF = mybir.ActivationFunctionType

    nc = bacc.Bacc(
        "TRN2", target_bir_lowering=False, debug=False,
        num_devices=(1 if single else NCORE),
    )

    # ---------------- DRAM parameters ----------------
    xT = nc.dram_tensor("xT", [DIN, TOK], BF, kind="ExternalInput")
    wqkv = nc.dram_tensor("wqkv", [DIN, 3 * DIM], BF, kind="ExternalInput")
    bqkv = nc.dram_tensor("bqkv", [1, 3 * DIM], BF, kind="ExternalInput")
    wsum = nc.dram_tensor("wsum", [DIN, 2], BF, kind="ExternalInput")
    tabs = nc.dram_tensor("tabs", [4, 128, TOK], BF, kind="ExternalInput")
    wout = nc.dram_tensor("wout", [DIM, DIN], BF, kind="ExternalInput")
    out = nc.dram_tensor("out", [DIN, TOK], F32, kind="ExternalOutput")
    dbgt = (nc.dram_tensor("dbg", [DIN, TOK], F32, kind="ExternalOutput")
            if dbg else None)

    # internal DRAM
    KSZ = 1024 * TOK + TOK            # k payload + rstd_k row per rank
    VSZ = 1024 * TOK                  # v payload
    agk_in = nc.dram_tensor("agk_in", [KSZ], BF)
    agk_out = nc.dram_tensor("agk_out", [4 * KSZ], BF)
    agv_in = nc.dram_tensor("agv_in", [VSZ], BF)
    agv_out = nc.dram_tensor("agv_out", [4 * VSZ], BF)

    RG = [[0, 1, 2, 3], [4, 5, 6, 7]]

    with tile.TileContext(nc) as tc, ExitStack() as CTX:
        # ---------------- persistent SBUF ----------------
        pp = CTX.enter_context(tc.tile_pool(name="persist", bufs=1))
        qhat = [pp.tile([128, 1024], BF, tag=f"qhat{c}", name=f"qhat{c}")
                for c in range(4)]
        ktf = [pp.tile([128, N], BF, tag=f"ktf{c}", name=f"ktf{c}")
               for c in range(8)]
        vaug = [pp.tile([128, 16 * 65], BF, tag=f"vaug{c}", name=f"vaug{c}")
                for c in range(KC)]
        attnT = [pp.tile([128, TOK], BF, tag=f"attnT{c}", name=f"attnT{c}")
                 for c in range(8)]
        xt_sb = pp.tile([128, 8, TOK], BF, tag="xt_sb", name="xt_sb")
        xt_c = [xt_sb[:, ci, :] for ci in range(8)]
        # rope tables, duplicated halves: [Cq|Cq], [Sq|Sq], [Ck|Ck], [Sk|Sk]
        tab_r = [pp.tile([128, 1024], BF, tag=f"tabr{j}", name=f"tabr{j}")
                 for j in range(4)]
        tab_qf = [pp.tile([128, 1024], BF, tag=f"tabqf{j}", name=f"tabqf{j}")
                  for j in range(2)]
        bias_sb = pp.tile([1, 3 * DIM], BF, tag="bias_sb", name="bias_sb")
        wsum_sb = pp.tile([128, 8, 2], BF, tag="wsum_sb", name="wsum_sb")
        rstdk_t = pp.tile([128, 16], F32, tag="rstdk_t", name="rstdk_t")
        ones_t = pp.tile([1, TOK], BF, tag="ones_t", name="ones_t")
        ones_c = pp.tile([1, 128], BF, tag="ones_c", name="ones_c")
        eps_sb = pp.tile([1, 1], F32, tag="eps_sb", name="eps_sb")

        nc.vector.memset(ones_t[:], 1.0)
        nc.vector.memset(ones_c[:], 1.0)
        nc.vector.memset(eps_sb[:], EPS)

        # ---------------- input DMAs ----------------
        for ci in range(8):
            nc.scalar.dma_start(
                out=xt_sb[:, ci, :],
                in_=xT.ap()[ci * 128:(ci + 1) * 128, :],
            )
        for j in range(4):
            for h in range(2):
                nc.gpsimd.dma_start(
                    out=tab_r[j][:, h * TOK:(h + 1) * TOK], in_=tabs.ap()[j])
        nc.gpsimd.dma_start(
            out=wsum_sb[:], in_=wsum.ap().rearrange("(c p) w -> p c w", p=128))
        if not zero_bias:
            nc.gpsimd.dma_start(out=bias_sb[:], in_=bqkv.ap())

        # ---------------- phase 1: qkv proj + norm + rope + AGs ----------
        p1 = ExitStack()
        wq_pool = p1.enter_context(tc.tile_pool(name="wq", bufs=3))
        ev_pool = p1.enter_context(tc.tile_pool(name="ev", bufs=4))
        sq_pool = p1.enter_context(tc.tile_pool(name="sq", bufs=2))
        kh_pool = p1.enter_context(tc.tile_pool(name="kh", bufs=1))
        rs_pool = p1.enter_context(tc.tile_pool(name="rs", bufs=2))
        # PSUM: PA (2x [128,1024] = 4 banks) lives for the whole kernel.
        PA = CTX.enter_context(tc.tile_pool(name="PA", bufs=2, space="PSUM"))
        ssqp = p1.enter_context(tc.tile_pool(name="ssqp", bufs=1, space="PSUM"))
        bcp = p1.enter_context(tc.tile_pool(name="bcp", bufs=1, space="PSUM"))

        def proj_tile(c4, which):
            """PA psum [128, 1024] = chunks (2c4, 2c4+1) of q/k, [chan, tok]."""
            co0 = which * 8 + 2 * c4
            ps = PA.tile([128, 1024], F32, tag="pa", name=f"proj{which}_{c4}")
            wt = wq_pool.tile([128, 8, 256], BF, tag="wt", name="wt")
            nc.sync.dma_start(
                out=wt[:],
                in_=wqkv.ap()[:, co0 * 128:(co0 + 2) * 128].rearrange(
                    "(c p) m -> p c m", p=128),
            )
            for half in range(2):
                for ci in range(8):
                    nc.tensor.matmul(
                        ps[:, half * TOK:(half + 1) * TOK],
                        wt[:, ci, half * 128:(half + 1) * 128],
                        xt_sb[:, ci, :],
                        start=(ci == 0), stop=(zero_bias and ci == 7),
                    )
                if not zero_bias:
                    co = co0 + half
                    nc.tensor.matmul(
                        ps[:, half * TOK:(half + 1) * TOK],
                        bias_sb[:, co * 128:(co + 1) * 128], ones_t[:],
                        start=False, stop=True,
                    )
            return ps

        def norm_rope_tensor(which, dst_tiles, tab_pair):
            """which: 0 q, 1 k. Per c4: proj, evict, sumsq, shuffle, rope.
            Returns the ssq psum row [1, TOK] (callers finish rstd)."""
            ssq = ssqp.tile([1, TOK], F32, tag="ssq", name=f"ssq{which}")
            qts = []
            for c4 in range(4):
                ps = proj_tile(c4, which)
                qt = ev_pool.tile([128, 1024], BF, tag="qt", name="qt")
                nc.vector.tensor_copy(qt[:], ps[:])
                sqt = sq_pool.tile([128, 1024], BF, tag="sqt", name="sqt")
                nc.gpsimd.tensor_mul(sqt[:], qt[:], qt[:])
                for half in range(2):
                    c = 2 * c4 + half
                    nc.tensor.matmul(
                        ssq[:], wsum_sb[:, c, which:which + 1],
                        sqt[:, half * TOK:(half + 1) * TOK],
                        start=(c == 0), stop=(c == 7),
                    )
                qts.append(qt)
            for c4 in range(4):
                qt = qts[c4]
                shuf = ev_pool.tile([128, 1024], BF, tag="shuf", name="shuf")
                nc.vector.stream_shuffle(shuf[:], qt[:], SWAP_MASK)
                m1 = sq_pool.tile([128, 1024], BF, tag="m1", name="m1")
                nc.vector.tensor_mul(m1[:], qt[:], tab_pair[0][:])
                nc.vector.tensor_mul(shuf[:], shuf[:], tab_pair[1][:])
                nc.vector.tensor_add(dst_tiles[c4][:], m1[:], shuf[:])
            return ssq

        # ---- K: proj + rope (unnormalized); rstd_k rides with AG-v.
        khat = [kh_pool.tile([128, 1024], BF, tag=f"khat{i}", name=f"khat{i}")
                for i in range(4)]
        ssq_k = norm_rope_tensor(1, khat, (tab_r[2], tab_r[3]))
        for c4 in range(4):
            # agk layout: [8 chan-chunks][128 chan][512 tok]
            dstap = bass.AP(
                tensor=agk_in.ap().tensor,
                offset=(2 * c4) * 128 * TOK,
                ap=[[TOK, 128], [128 * TOK, 2], [1, TOK]],
            )
            nc.gpsimd.dma_start(out=dstap, in_=khat[c4][:].rearrange(
                "p (h t) -> p h t", h=2))
        # rstd_k = 1/sqrt(var+eps), rides in the AG-k payload tail
        sqs_k = rs_pool.tile([1, TOK], F32, tag="sqs", name="sqs_k")
        nc.scalar.activation(sqs_k[:], ssq_k[:], AF.Sqrt, bias=eps_sb[:])
        rstd_k = rs_pool.tile([1, TOK], BF, tag="rstd", name="rstd_k")
        with nc.allow_low_precision(reason="bf16 rstd is sufficient"):
            nc.vector.reciprocal(out=rstd_k[:], in_=sqs_k[:])
        nc.gpsimd.dma_start(
            out=bass.AP(tensor=agk_in.ap().tensor, offset=1024 * TOK,
                        ap=[[1, TOK]]),
            in_=rstd_k[:],
        )
        if single:
            for r in range(4):
                nc.gpsimd.dma_start(
                    out=bass.AP(tensor=agk_out.ap().tensor, offset=r * KSZ,
                                ap=[[1, KSZ]]),
                    in_=bass.AP(tensor=agk_in.ap().tensor, offset=0,
                                ap=[[1, KSZ]]),
                )
        else:
            nc.gpsimd.collective_compute(
                "AllGather", mybir.AluOpType.bypass, replica_groups=RG,
                ins=[agk_in.ap().opt()], outs=[agk_out.ap().opt()],
            )

        # ---- V: natural [tok, chan] layout; 4 psum tiles of [128 tok, 1024]
        vw_pool = p1.enter_context(tc.tile_pool(name="vw", bufs=1))
        vw = vw_pool.tile([128, 8, 1024], BF, tag="vw", name="vw")
        for ci in range(8):
            nc.sync.dma_start(
                out=vw[:, ci, :],
                in_=wqkv.ap()[ci * 128:(ci + 1) * 128, 2 * DIM:],
            )
        for t4 in range(4):
            ps = PA.tile([128, 1024], F32, tag="pa", name=f"vproj{t4}")
            for half in range(2):
                for ci in range(8):
                    nc.tensor.matmul(
                        ps[:, half * TOK:(half + 1) * TOK],
                        xt_sb[:, ci, t4 * 128:(t4 + 1) * 128],
                        vw[:, ci, half * TOK:(half + 1) * TOK],
                        start=(ci == 0), stop=(zero_bias and ci == 7),
                    )
                if not zero_bias:
                    nc.tensor.matmul(
                        ps[:, half * TOK:(half + 1) * TOK], ones_c[:],
                        bias_sb[:, 2 * DIM + half * TOK:
                                2 * DIM + (half + 1) * TOK],
                        start=False, stop=True,
                    )
            vl = ev_pool.tile([128, 1024], BF, tag="qt", name=f"vloc{t4}")
            nc.vector.tensor_copy(vl[:], ps[:])
            # agv layout: [4 t4][128 tok][1024 chan]
            nc.sync.dma_start(
                out=bass.AP(tensor=agv_in.ap().tensor,
                            offset=t4 * 128 * 1024,
                            ap=[[1024, 128], [1, 1024]]),
                in_=vl[:],
            )
        if single:
            for r in range(4):
                nc.gpsimd.dma_start(
                    out=bass.AP(tensor=agv_out.ap().tensor, offset=r * VSZ,
                                ap=[[1, VSZ]]),
                    in_=bass.AP(tensor=agv_in.ap().tensor, offset=0,
                                ap=[[1, VSZ]]),
                )
        else:
            nc.gpsimd.collective_compute(
                "AllGather", mybir.AluOpType.bypass, replica_groups=RG,
                ins=[agv_in.ap().opt()], outs=[agv_out.ap().opt()],
            )

        # ---- Q: proj + norm (bc matmul folds rstd_q into tabs) + rope
        ssq_q = ssqp.tile([1, TOK], F32, tag="ssq", name="ssq_q")
        qts = []
        for c4 in range(4):
            ps = proj_tile(c4, 0)
            qt = ev_pool.tile([128, 1024], BF, tag="qt", name="qt")
            nc.vector.tensor_copy(qt[:], ps[:])
            sqt = sq_pool.tile([128, 1024], BF, tag="sqt", name="sqt")
            nc.gpsimd.tensor_mul(sqt[:], qt[:], qt[:])
            for half in range(2):
                c = 2 * c4 + half
                nc.tensor.matmul(
                    ssq_q[:], wsum_sb[:, c, 0:1],
                    sqt[:, half * TOK:(half + 1) * TOK],
                    start=(c == 0), stop=(c == 7),
                )
            qts.append(qt)
        sqs_q = rs_pool.tile([1, TOK], F32, tag="sqs", name="sqs_q")
        nc.scalar.activation(sqs_q[:], ssq_q[:], AF.Sqrt, bias=eps_sb[:])
        rstd_q = rs_pool.tile([1, TOK], BF, tag="rstd", name="rstd_q")
        with nc.allow_low_precision(reason="bf16 rstd is sufficient"):
            nc.vector.reciprocal(out=rstd_q[:], in_=sqs_q[:])
        bc = bcp.tile([128, TOK], F32, tag="bc", name="bc")
        nc.tensor.matmul(bc[:], ones_c[:], rstd_q[:], start=True, stop=True)
        for j in range(2):
            for half in range(2):
                nc.vector.tensor_mul(
                    tab_qf[j][:, half * TOK:(half + 1) * TOK],
                    tab_r[j][:, half * TOK:(half + 1) * TOK], bc[:])
        for c4 in range(4):
            qt = qts[c4]
            shuf = ev_pool.tile([128, 1024], BF, tag="shuf", name="shuf")
            nc.vector.stream_shuffle(shuf[:], qt[:], SWAP_MASK)
            m1 = sq_pool.tile([128, 1024], BF, tag="m1", name="m1")
            nc.vector.tensor_mul(m1[:], qt[:], tab_qf[0][:])
            nc.vector.tensor_mul(shuf[:], shuf[:], tab_qf[1][:])
            nc.vector.tensor_add(qhat[c4][:], m1[:], shuf[:])

        # ---- post-AG loads (uniform across ranks; includes own block)
        ld_engines = [nc.sync, nc.scalar]
        for c in range(8):
            for r in range(4):
                srcap = bass.AP(
                    tensor=agk_out.ap().tensor,
                    offset=r * KSZ + c * 128 * TOK,
                    ap=[[TOK, 128], [1, TOK]],
                )
                eng = ld_engines[(c * 4 + r) % 2]
                eng.dma_start(
                    out=ktf[c][:, r * TOK:(r + 1) * TOK], in_=srcap)
        for kc in range(KC):
            dst = vaug[kc][:].rearrange("p (h c) -> p h c", c=65)
            nc.vector.memset(dst[:, :, 64:65], 1.0)
        for r in range(4):
            nc.gpsimd.dma_start(
                out=rstdk_t[:, r * 4:(r + 1) * 4],
                in_=bass.AP(tensor=agk_out.ap().tensor,
                            offset=r * KSZ + 1024 * TOK,
                            ap=[[1, 128], [128, 4]]),
            )
            for t4 in range(4):
                kc = r * 4 + t4
                srcap = bass.AP(
                    tensor=agv_out.ap().tensor,
                    offset=r * VSZ + t4 * 128 * 1024,
                    ap=[[1024, 128], [64, 16], [1, 64]],
                )
                dst = vaug[kc][:].rearrange("p (h c) -> p h c", c=65)
                veng = [nc.sync, nc.scalar][kc % 2]
                veng.dma_start(out=dst[:, :, 0:64], in_=srcap)

        def dump8(tiles, cols=None):
            dmp = ExitStack()
            dp = dmp.enter_context(tc.tile_pool(name="dump", bufs=2))
            for c, t in enumerate(tiles[:8]):
                ncols = min(TOK, t.shape[-1] if cols is None else TOK)
                f = dp.tile([t.shape[0], ncols], F32, tag="dmp", name="dmp")
                srcap = t[:, cols] if cols is not None else t[:]
                nc.vector.tensor_copy(f[:], srcap)
                nc.gpsimd.dma_start(
                    out=dbgt.ap()[c * 128:c * 128 + t.shape[0], :ncols],
                    in_=f[:])
            dmp.close()

        if dbg == "khat":
            dump8(khat, cols=slice(0, TOK))
        if dbg == "qhat":
            dump8(qhat, cols=slice(0, TOK))
        if dbg == "ktf":
            dump8(ktf, cols=slice(0, TOK))
        if dbg == "vaug":
            dump8(vaug[:8], cols=slice(0, TOK))

        p1.close()

        # ---------------- phase 2: attention ----------------
        wo_pool = CTX.enter_context(tc.tile_pool(name="wo", bufs=1))
        p2 = ExitStack()
        avp = p2.enter_context(tc.tile_pool(name="avp", bufs=2, space="PSUM"))
        expt_pool = p2.enter_context(tc.tile_pool(name="expt", bufs=16))
        nrm_pool = p2.enter_context(tc.tile_pool(name="nrm", bufs=2))

        expt = {}
        av_tiles = {}

        def emit_scores_exp(hg, kc):
            """sc psum [128 ktok, 2 heads x 512 qtok] -> exp w/ rstd_k scale."""
            sc = PA.tile([128, 1024], F32, tag="pa", name=f"sc{hg}_{kc}")
            c4, half = hg // 2, hg % 2
            for hh in range(2):
                nc.tensor.matmul(
                    sc[:, hh * TOK:(hh + 1) * TOK],
                    ktf[hg][hh * 64:hh * 64 + 64, kc * 128:(kc + 1) * 128],
                    qhat[c4][hh * 64:hh * 64 + 64,
                             half * TOK:(half + 1) * TOK],
                    start=True, stop=True,
                )
            e = expt_pool.tile([128, 1024], BF, tag="expt", name="expt")
            nc.scalar.activation(e[:], sc[:], AF.Exp,
                                 scale=rstdk_t[:, kc:kc + 1])
            expt[(hg, kc)] = e

        def emit_av(hg, kc):
            if kc == 0:
                av_tiles[hg] = avp.tile([65, 1024], F32, tag="av", name="av")
            av = av_tiles[hg]
            for hh in range(2):
                h = hg * 2 + hh
                nc.tensor.matmul(
                    av[:, hh * TOK:(hh + 1) * TOK],
                    vaug[kc][:, h * 65:(h + 1) * 65],
                    expt[(hg, kc)][:, hh * TOK:(hh + 1) * TOK],
                    start=(kc == 0), stop=(kc == KC - 1),
                )
            del expt[(hg, kc)]

        def emit_normalize(hg):
            av = av_tiles.pop(hg)
            rec = nrm_pool.tile([1, 1024], BF, tag="rec", name="rec")
            with nc.allow_low_precision(reason="bf16 softmax recip ok"):
                nc.vector.reciprocal(out=rec[:], in_=av[64:65, :])
            rb = nrm_pool.tile([64, 1024], BF, tag="rb", name="rb")
            nc.gpsimd.partition_broadcast(rb[:], rec[:])
            for hh in range(2):
                nc.vector.tensor_mul(
                    attnT[hg][hh * 64:hh * 64 + 64, :],
                    av[0:64, hh * TOK:(hh + 1) * TOK],
                    rb[:, hh * TOK:(hh + 1) * TOK],
                )

        for hg in range(9):
            for kc in range(KC):
                if hg >= 1:
                    emit_av(hg - 1, kc)
                if hg < 8:
                    emit_scores_exp(hg, kc)
            if hg == 5:
                wo_res = wo_pool.tile([128, 8, 1024], BF, tag="wores",
                                      name="wores")
                for ci in range(8):
                    nc.sync.dma_start(
                        out=wo_res[:, ci, :],
                        in_=wout.ap()[ci * 128:(ci + 1) * 128, :],
                    )
            if hg >= 1:
                emit_normalize(hg - 1)
            if dbg == "expt0" and hg == 0:
                dump8([expt[(0, kc)] for kc in range(8)], cols=slice(0, TOK))

        p2.close()

        # ---------------- phase 3: output projection ----------------
        p3 = ExitStack()
        ob_pool = p3.enter_context(tc.tile_pool(name="ob", bufs=2))
        for pair in range(4):
            ps = PA.tile([128, 1024], F32, tag="pa", name=f"outp{pair}")
            for half in range(2):
                co = 2 * pair + half
                for ci in range(8):
                    nc.tensor.matmul(
                        ps[:, half * TOK:(half + 1) * TOK],
                        wo_res[:, ci, co * 128:(co + 1) * 128],
                        attnT[ci][:],
                        start=(ci == 0), stop=(ci == 7),
                    )
            osb = ob_pool.tile([128, 1024], F32, tag="osb", name="osb")
            nc.vector.tensor_copy(osb[:], ps[:])
            oeng = [nc.sync, nc.scalar, nc.gpsimd, nc.sync][pair]
            oeng.dma_start(
                out=bass.AP(tensor=out.ap().tensor,
                            offset=(2 * pair) * 128 * TOK,
                            ap=[[TOK, 128], [128 * TOK, 2], [1, TOK]]),
                in_=osb[:].rearrange("p (h t) -> p h t", h=2),
            )
        p3.close()

    nc.compile()
    return nc


def _host_prep(inputs):
    import ml_dtypes

    bf16 = ml_dtypes.bfloat16
    x = np.asarray(inputs["x"], np.float32)
    Wqkv = np.asarray(inputs["Wqkv"], np.float32)
    bqkv = np.asarray(inputs["bqkv"], np.float32)
    qs = np.asarray(inputs["q_scale"], np.float32)
    ks = np.asarray(inputs["k_scale"], np.float32)
    Wout = np.asarray(inputs["Wout"], np.float32)

    # per-head channel order: evens of pairs 0-15, odds of pairs 0-15,
    # evens of pairs 16-31, odds of pairs 16-31 (so the rope x1<->x2 swap is
    # a +-16-partition shuffle within 32-blocks, expressible on the DVE)
    p64 = np.concatenate([
        np.arange(0, 32, 2), np.arange(1, 32, 2),
        np.arange(32, 64, 2), np.arange(33, 64, 2),
    ])
    perm = np.concatenate([64 * h + p64 for h in range(H)])

    qsp, ksp = qs[perm], ks[perm]
    Wq = Wqkv[:, :DIM][:, perm] * qsp[None, :]
    Wk = Wqkv[:, DIM:2 * DIM][:, perm] * ksp[None, :]
    Wv = Wqkv[:, 2 * DIM:]
    W = np.concatenate([Wq, Wk, Wv], 1).astype(bf16)
    bq = bqkv[:DIM][perm] * qsp
    bk = bqkv[DIM:2 * DIM][perm] * ksp
    bias = np.concatenate([bq, bk, bqkv[2 * DIM:]])[None, :].astype(bf16)
    wsum = np.stack(
        [1.0 / (DIM * qsp ** 2), 1.0 / (DIM * ksp ** 2)], 1
    ).astype(bf16)

    inv_freq = 1.0 / (BASE ** (np.arange(0, HD, 2).astype(np.float32) / HD))
    pos = np.maximum(np.arange(N) - 1, 0).astype(np.float32)
    ang = pos[:, None] * inv_freq[None, :]
    cosT, sinT = np.cos(ang).T, np.sin(ang).T           # (32, N)
    C64 = np.concatenate([cosT[0:16], cosT[0:16], cosT[16:32], cosT[16:32]], 0)
    S64 = np.concatenate([-sinT[0:16], sinT[0:16], -sinT[16:32], sinT[16:32]], 0)
    C128 = np.tile(C64, (2, 1))                          # (128, N)
    S128 = np.tile(S64, (2, 1))

    in_maps = []
    for core in range(NCORE):
        b, sh = core // 4, core % 4
        t0 = sh * TOK
        xTs = np.ascontiguousarray(x[b, t0:t0 + TOK, :].T).astype(bf16)
        tabs = np.stack([
            C128[:, t0:t0 + TOK] * 0.125,
            S128[:, t0:t0 + TOK] * 0.125,
            C128[:, t0:t0 + TOK],
            S128[:, t0:t0 + TOK],
        ]).astype(bf16)
        in_maps.append({
            "xT": xTs,
            "wqkv": W,
            "bqkv": bias,
            "wsum": wsum,
            "tabs": np.ascontiguousarray(tabs),
            "wout": Wout.astype(bf16),
        })
    return in_maps


LAST_EXEC_NS = None


def kernel(**inputs):
    global LAST_EXEC_NS
    import os
    from concourse.bass_utils import run_bass_kernel_spmd

    dbg = os.environ.get("KERNEL_DBG") or None
    zb = bool(np.all(np.asarray(inputs["bqkv"]) == 0))
    key = f"nc{dbg}{zb}"
    if key not in _CACHE:
        _CACHE[key] = _build_nc(dbg, zero_bias=zb)
    nc = _CACHE[key]

    in_maps = _host_prep(inputs)
    trace = bool(int(os.environ.get("KERNEL_TRACE", "0")))
    tmpdir = None
    if trace:
        import tempfile
        import concourse.bass_utils as _bu
        _bu.upload_artifacts = lambda d: d  # keep artifacts local
        tmpdir = tempfile.mkdtemp(prefix="ktrace_")
        print("TRACE DIR:", tmpdir)
    res = run_bass_kernel_spmd(
        nc, in_maps, core_ids=list(range(NCORE)), trace=trace, tmpdir=tmpdir
    )
    LAST_EXEC_NS = res.exec_time_ns
    bout = np.asarray(inputs["bout"], np.float32)
    out = np.empty((B, N, DIN), np.float32)
    for core in range(NCORE):
        b, sh = core // 4, core % 4
        t0 = sh * TOK
        out[b, t0:t0 + TOK, :] = res.results[core]["out"].T
    out += bout[None, None, :]
    return out


def kernel_raw(inputs):
    """Debug helper: run and return the per-core raw [1024, 512] outputs."""
    global LAST_EXEC_NS
    import os
    from concourse.bass_utils import run_bass_kernel_spmd

    dbg = os.environ.get("KERNEL_DBG") or None
    zb = bool(np.all(np.asarray(inputs["bqkv"]) == 0))
    key = f"nc{dbg}{zb}"
    if key not in _CACHE:
        _CACHE[key] = _build_nc(dbg, zero_bias=zb)
    nc = _CACHE[key]
    in_maps = _host_prep(inputs)
    res = run_bass_kernel_spmd(nc, in_maps, core_ids=list(range(NCORE)))
    LAST_EXEC_NS = res.exec_time_ns
    key = "dbg" if dbg else "out"
    return [r[key] for r in res.results]


# revision 18
# speedup vs baseline: 1.1085x; 1.1085x over previous
"""Trainium2 8-core Bass kernel for nn_Attention_7112465842253.

Token-sharded attention: 512 tokens/core (cores 0-3 = batch 0, 4-7 = batch 1).
Per core: QKV projection in bf16 (q/k in transposed [chan, tok] layout, v in
natural [tok, chan]), RMSNorm via weighted-sumsq matmul; the k-side rstd is
NOT applied to k at all -- it rides the AllGather as a 512-element row and is
folded into the softmax exp as a per-partition (per-k-token) activation
scale. RoPE as x*C + shuffle(x)*S with a DVE stream_shuffle partition swap.
Two AllGathers (k first, then v+rstd_k) inside each 4-core batch group,
non-causal attention in scoresT layout (k-tokens on partitions; softmax
denominator from a ones column appended to V; Exp fused into the PSUM->SBUF
eviction on ScalarE with the rstd_k scale), normalization via DVE reciprocal
+ gpsimd partition_broadcast (no DRAM round trips), then the output
projection. Host does layout prep and reassembly.
"""

import numpy as np

B, N, DIN, DIM, H, HD = 2, 2048, 1024, 1024, 16, 64
NCORE = 8
TOK = 512
EPS = 1e-6
BASE = 10000.0
KC = N // 128        # 16 k-token chunks

_CACHE = {}

# stream_shuffle mask: per-partition permutation applied within each
# 32-partition block; swaps 16-halves (rope x1/x2 blocks) within 32-blocks
SWAP_MASK = list(range(16, 32)) + list(range(0, 16))


def _build_nc(dbg=None, single=False, zero_bias=False):
    import concourse.bass as bass
    import concourse.tile as tile
    from concourse import bacc, mybir
    from contextlib import ExitStack

    BF = mybir.dt.bfloat16
    F32 = mybir.dt.float32
    AF = mybir.ActivationFunctionType

    nc = bacc.Bacc(
        "TRN2", target_bir_lowering=False, debug=False,
        num_devices=(1 if single else NCORE),
    )

    # ---------------- DRAM parameters ----------------
    xT = nc.dram_tensor("xT", [DIN, TOK], BF, kind="ExternalInput")
    wqkv = nc.dram_tensor("wqkv", [DIN, 3 * DIM], BF, kind="ExternalInput")
    bqkv = nc.dram_tensor("bqkv", [1, 3 * DIM], BF, kind="ExternalInput")
    wsum = nc.dram_tensor("wsum", [DIN, 2], BF, kind="ExternalInput")
    tabs = nc.dram_tensor("tabs", [4, 128, TOK], BF, kind="ExternalInput")
    wout = nc.dram_tensor("wout", [DIM, DIN], BF, kind="ExternalInput")
    out = nc.dram_tensor("out", [DIN, TOK], F32, kind="ExternalOutput")
    dbgt = (nc.dram_tensor("dbg", [DIN, TOK], F32, kind="ExternalOutput")
            if dbg else None)

    # internal DRAM
    KSZ = 1024 * TOK + TOK            # k payload + rstd_k row per rank
    VSZ = 1024 * TOK                  # v payload
    agk_in = nc.dram_tensor("agk_in", [KSZ], BF)
    agk_out = nc.dram_tensor("agk_out", [4 * KSZ], BF)
    agv_in = nc.dram_tensor("agv_in", [VSZ], BF)
    agv_out = nc.dram_tensor("agv_out", [4 * VSZ], BF)

    RG = [[0, 1, 2, 3], [4, 5, 6, 7]]

    with tile.TileContext(nc) as tc, ExitStack() as CTX:
        # ---------------- persistent SBUF ----------------
        pp = CTX.enter_context(tc.tile_pool(name="persist", bufs=1))
        qhat = [pp.tile([128, 1024], BF, tag=f"qhat{c}", name=f"qhat{c}")
                for c in range(4)]
        ktf = [pp.tile([128, N], BF, tag=f"ktf{c}", name=f"ktf{c}")
               for c in range(8)]
        vaug = [pp.tile([128, 16 * 65], BF, tag=f"vaug{c}", name=f"vaug{c}")
                for c in range(KC)]
        attnT = [pp.tile([128, TOK], BF, tag=f"attnT{c}", name=f"attnT{c}")
                 for c in range(8)]
        xt_sb = pp.tile([128, 8, TOK], BF, tag="xt_sb", name="xt_sb")
        xt_c = [xt_sb[:, ci, :] for ci in range(8)]
        # rope tables, duplicated halves: [Cq|Cq], [Sq|Sq], [Ck|Ck], [Sk|Sk]
        tab_r = [pp.tile([128, 1024], BF, tag=f"tabr{j}", name=f"tabr{j}")
                 for j in range(4)]
        tab_qf = [pp.tile([128, 1024], BF, tag=f"tabqf{j}", name=f"tabqf{j}")
                  for j in range(2)]
        bias_sb = pp.tile([1, 3 * DIM], BF, tag="bias_sb", name="bias_sb")
        wsum_sb = pp.tile([128, 8, 2], BF, tag="wsum_sb", name="wsum_sb")
        rstdk_t = pp.tile([128, 16], F32, tag="rstdk_t", name="rstdk_t")
        ones_t = pp.tile([1, TOK], BF, tag="ones_t", name="ones_t")
        ones_c = pp.tile([1, 128], BF, tag="ones_c", name="ones_c")
        eps_sb = pp.tile([1, 1], F32, tag="eps_sb", name="eps_sb")

        for _kc in range(KC):
            _d = vaug[_kc][:].rearrange("p (h c) -> p h c", c=65)
            nc.vector.memset(_d[:, :, 64:65], 1.0)
        nc.vector.memset(ones_t[:], 1.0)
        nc.vector.memset(ones_c[:], 1.0)
        nc.vector.memset(eps_sb[:], EPS)

        # ---------------- input DMAs ----------------
        for ci in range(8):
            nc.scalar.dma_start(
                out=xt_sb[:, ci, :],
                in_=xT.ap()[ci * 128:(ci + 1) * 128, :],
            )
        for j in range(4):
            for h in range(2):
                nc.gpsimd.dma_start(
                    out=tab_r[j][:, h * TOK:(h + 1) * TOK], in_=tabs.ap()[j])
        nc.gpsimd.dma_start(
            out=wsum_sb[:], in_=wsum.ap().rearrange("(c p) w -> p c w", p=128))
        if not zero_bias:
            nc.gpsimd.dma_start(out=bias_sb[:], in_=bqkv.ap())

        # ---------------- phase 1: qkv proj + norm + rope + AGs ----------
        p1 = ExitStack()
        wq_pool = p1.enter_context(tc.tile_pool(name="wq", bufs=3))
        ev_pool = p1.enter_context(tc.tile_pool(name="ev", bufs=4))
        sq_pool = p1.enter_context(tc.tile_pool(name="sq", bufs=2))
        kh_pool = p1.enter_context(tc.tile_pool(name="kh", bufs=1))
        rs_pool = p1.enter_context(tc.tile_pool(name="rs", bufs=2))
        # PSUM: PA (2x [128,1024] = 4 banks) lives for the whole kernel.
        PA = CTX.enter_context(tc.tile_pool(name="PA", bufs=2, space="PSUM"))
        ssqp = p1.enter_context(tc.tile_pool(name="ssqp", bufs=1, space="PSUM"))
        bcp = p1.enter_context(tc.tile_pool(name="bcp", bufs=1, space="PSUM"))

        def proj_tile(c4, which):
            """PA psum [128, 1024] = chunks (2c4, 2c4+1) of q/k, [chan, tok]."""
            co0 = which * 8 + 2 * c4
            ps = PA.tile([128, 1024], F32, tag="pa", name=f"proj{which}_{c4}")
            wt = wq_pool.tile([128, 8, 256], BF, tag="wt", name="wt")
            nc.sync.dma_start(
                out=wt[:],
                in_=wqkv.ap()[:, co0 * 128:(co0 + 2) * 128].rearrange(
                    "(c p) m -> p c m", p=128),
            )
            for half in range(2):
                for ci in range(8):
                    nc.tensor.matmul(
                        ps[:, half * TOK:(half + 1) * TOK],
                        wt[:, ci, half * 128:(half + 1) * 128],
                        xt_sb[:, ci, :],
                        start=(ci == 0), stop=(zero_bias and ci == 7),
                    )
                if not zero_bias:
                    co = co0 + half
                    nc.tensor.matmul(
                        ps[:, half * TOK:(half + 1) * TOK],
                        bias_sb[:, co * 128:(co + 1) * 128], ones_t[:],
                        start=False, stop=True,
                    )
            return ps

        def norm_rope_tensor(which, dst_tiles, tab_pair):
            """which: 0 q, 1 k. Per c4: proj, evict, sumsq, shuffle, rope.
            Returns the ssq psum row [1, TOK] (callers finish rstd)."""
            ssq = ssqp.tile([1, TOK], F32, tag="ssq", name=f"ssq{which}")
            qts = []
            for c4 in range(4):
                ps = proj_tile(c4, which)
                qt = ev_pool.tile([128, 1024], BF, tag="qt", name="qt")
                nc.vector.tensor_copy(qt[:], ps[:])
                sqt = sq_pool.tile([128, 1024], BF, tag="sqt", name="sqt")
                nc.vector.tensor_mul(sqt[:], qt[:], qt[:])
                for half in range(2):
                    c = 2 * c4 + half
                    nc.tensor.matmul(
                        ssq[:], wsum_sb[:, c, which:which + 1],
                        sqt[:, half * TOK:(half + 1) * TOK],
                        start=(c == 0), stop=(c == 7),
                    )
                qts.append(qt)
            for c4 in range(4):
                qt = qts[c4]
                shuf = ev_pool.tile([128, 1024], BF, tag="shuf", name="shuf")
                nc.vector.stream_shuffle(shuf[:], qt[:], SWAP_MASK)
                m1 = sq_pool.tile([128, 1024], BF, tag="m1", name="m1")
                nc.vector.tensor_mul(m1[:], qt[:], tab_pair[0][:])
                nc.vector.tensor_mul(shuf[:], shuf[:], tab_pair[1][:])
                nc.vector.tensor_add(dst_tiles[c4][:], m1[:], shuf[:])
            return ssq

        # ---- K: proj + rope (unnormalized); rstd_k rides with AG-v.
        khat = [kh_pool.tile([128, 1024], BF, tag=f"khat{i}", name=f"khat{i}")
                for i in range(4)]
        ssq_k = norm_rope_tensor(1, khat, (tab_r[2], tab_r[3]))
        for c4 in range(4):
            # agk layout: [8 chan-chunks][128 chan][512 tok]
            dstap = bass.AP(
                tensor=agk_in.ap().tensor,
                offset=(2 * c4) * 128 * TOK,
                ap=[[TOK, 128], [128 * TOK, 2], [1, TOK]],
            )
            nc.gpsimd.dma_start(out=dstap, in_=khat[c4][:].rearrange(
                "p (h t) -> p h t", h=2))
        # rstd_k = 1/sqrt(var+eps), rides in the AG-k payload tail
        sqs_k = rs_pool.tile([1, TOK], F32, tag="sqs", name="sqs_k")
        nc.scalar.activation(sqs_k[:], ssq_k[:], AF.Sqrt, bias=eps_sb[:])
        rstd_k = rs_pool.tile([1, TOK], BF, tag="rstd", name="rstd_k")
        with nc.allow_low_precision(reason="bf16 rstd is sufficient"):
            nc.vector.reciprocal(out=rstd_k[:], in_=sqs_k[:])
        nc.gpsimd.dma_start(
            out=bass.AP(tensor=agk_in.ap().tensor, offset=1024 * TOK,
                        ap=[[1, TOK]]),
            in_=rstd_k[:],
        )
        if single:
            for r in range(4):
                nc.gpsimd.dma_start(
                    out=bass.AP(tensor=agk_out.ap().tensor, offset=r * KSZ,
                                ap=[[1, KSZ]]),
                    in_=bass.AP(tensor=agk_in.ap().tensor, offset=0,
                                ap=[[1, KSZ]]),
                )
        else:
            nc.gpsimd.collective_compute(
                "AllGather", mybir.AluOpType.bypass, replica_groups=RG,
                ins=[agk_in.ap().opt()], outs=[agk_out.ap().opt()],
            )
        for r in range(4):
            nc.gpsimd.dma_start(
                out=rstdk_t[:, r * 4:(r + 1) * 4],
                in_=bass.AP(tensor=agk_out.ap().tensor,
                            offset=r * KSZ + 1024 * TOK,
                            ap=[[1, 128], [128, 4]]),
            )

        # ---- V: natural [tok, chan] layout; 4 psum tiles of [128 tok, 1024]
        vw_pool = p1.enter_context(tc.tile_pool(name="vw", bufs=1))
        vw = vw_pool.tile([128, 8, 1024], BF, tag="vw", name="vw")
        for ci in range(8):
            nc.sync.dma_start(
                out=vw[:, ci, :],
                in_=wqkv.ap()[ci * 128:(ci + 1) * 128, 2 * DIM:],
            )
        for t4 in range(4):
            ps = PA.tile([128, 1024], F32, tag="pa", name=f"vproj{t4}")
            for half in range(2):
                for ci in range(8):
                    nc.tensor.matmul(
                        ps[:, half * TOK:(half + 1) * TOK],
                        xt_sb[:, ci, t4 * 128:(t4 + 1) * 128],
                        vw[:, ci, half * TOK:(half + 1) * TOK],
                        start=(ci == 0), stop=(zero_bias and ci == 7),
                    )
                if not zero_bias:
                    nc.tensor.matmul(
                        ps[:, half * TOK:(half + 1) * TOK], ones_c[:],
                        bias_sb[:, 2 * DIM + half * TOK:
                                2 * DIM + (half + 1) * TOK],
                        start=False, stop=True,
                    )
            vl = ev_pool.tile([128, 1024], BF, tag="qt", name=f"vloc{t4}")
            nc.vector.tensor_copy(vl[:], ps[:])
            # agv layout: [4 t4][128 tok][1024 chan]
            nc.sync.dma_start(
                out=bass.AP(tensor=agv_in.ap().tensor,
                            offset=t4 * 128 * 1024,
                            ap=[[1024, 128], [1, 1024]]),
                in_=vl[:],
            )
        if single:
            for r in range(4):
                nc.gpsimd.dma_start(
                    out=bass.AP(tensor=agv_out.ap().tensor, offset=r * VSZ,
                                ap=[[1, VSZ]]),
                    in_=bass.AP(tensor=agv_in.ap().tensor, offset=0,
                                ap=[[1, VSZ]]),
                )
        else:
            nc.gpsimd.collective_compute(
                "AllGather", mybir.AluOpType.bypass, replica_groups=RG,
                ins=[agv_in.ap().opt()], outs=[agv_out.ap().opt()],
            )

        # ---- Q: proj + norm (bc matmul folds rstd_q into tabs) + rope
        ssq_q = ssqp.tile([1, TOK], F32, tag="ssq", name="ssq_q")
        qts = []
        for c4 in range(4):
            ps = proj_tile(c4, 0)
            qt = ev_pool.tile([128, 1024], BF, tag="qt", name="qt")
            nc.vector.tensor_copy(qt[:], ps[:])
            sqt = sq_pool.tile([128, 1024], BF, tag="sqt", name="sqt")
            nc.vector.tensor_mul(sqt[:], qt[:], qt[:])
            for half in range(2):
                c = 2 * c4 + half
                nc.tensor.matmul(
                    ssq_q[:], wsum_sb[:, c, 0:1],
                    sqt[:, half * TOK:(half + 1) * TOK],
                    start=(c == 0), stop=(c == 7),
                )
            qts.append(qt)
        sqs_q = rs_pool.tile([1, TOK], F32, tag="sqs", name="sqs_q")
        nc.scalar.activation(sqs_q[:], ssq_q[:], AF.Sqrt, bias=eps_sb[:])
        rstd_q = rs_pool.tile([1, TOK], BF, tag="rstd", name="rstd_q")
        with nc.allow_low_precision(reason="bf16 rstd is sufficient"):
            nc.vector.reciprocal(out=rstd_q[:], in_=sqs_q[:])
        bc = bcp.tile([128, TOK], F32, tag="bc", name="bc")
        nc.tensor.matmul(bc[:], ones_c[:], rstd_q[:], start=True, stop=True)
        for j in range(2):
            for half in range(2):
                nc.vector.tensor_mul(
                    tab_qf[j][:, half * TOK:(half + 1) * TOK],
                    tab_r[j][:, half * TOK:(half + 1) * TOK], bc[:])
        for c4 in range(4):
            qt = qts[c4]
            shuf = ev_pool.tile([128, 1024], BF, tag="shuf", name="shuf")
            nc.vector.stream_shuffle(shuf[:], qt[:], SWAP_MASK)
            m1 = sq_pool.tile([128, 1024], BF, tag="m1", name="m1")
            nc.vector.tensor_mul(m1[:], qt[:], tab_qf[0][:])
            nc.vector.tensor_mul(shuf[:], shuf[:], tab_qf[1][:])
            nc.vector.tensor_add(qhat[c4][:], m1[:], shuf[:])

        # ---- post-AG loads (uniform across ranks; includes own block)
        for c in range(8):
            for r in range(4):
                srcap = bass.AP(
                    tensor=agk_out.ap().tensor,
                    offset=r * KSZ + c * 128 * TOK,
                    ap=[[TOK, 128], [1, TOK]],
                )
                nc.sync.dma_start(
                    out=ktf[c][:, r * TOK:(r + 1) * TOK], in_=srcap)
        for r in range(4):
            for t4 in range(4):
                kc = r * 4 + t4
                srcap = bass.AP(
                    tensor=agv_out.ap().tensor,
                    offset=r * VSZ + t4 * 128 * 1024,
                    ap=[[1024, 128], [64, 16], [1, 64]],
                )
                dst = vaug[kc][:].rearrange("p (h c) -> p h c", c=65)
                nc.sync.dma_start(out=dst[:, :, 0:64], in_=srcap)

        def dump8(tiles, cols=None):
            dmp = ExitStack()
            dp = dmp.enter_context(tc.tile_pool(name="dump", bufs=2))
            for c, t in enumerate(tiles[:8]):
                ncols = min(TOK, t.shape[-1] if cols is None else TOK)
                f = dp.tile([t.shape[0], ncols], F32, tag="dmp", name="dmp")
                srcap = t[:, cols] if cols is not None else t[:]
                nc.vector.tensor_copy(f[:], srcap)
                nc.gpsimd.dma_start(
                    out=dbgt.ap()[c * 128:c * 128 + t.shape[0], :ncols],
                    in_=f[:])
            dmp.close()

        if dbg == "khat":
            dump8(khat, cols=slice(0, TOK))
        if dbg == "qhat":
            dump8(qhat, cols=slice(0, TOK))
        if dbg == "ktf":
            dump8(ktf, cols=slice(0, TOK))
        if dbg == "vaug":
            dump8(vaug[:8], cols=slice(0, TOK))

        p1.close()

        # ---------------- phase 2: attention ----------------
        wo_pool = CTX.enter_context(tc.tile_pool(name="wo", bufs=1))
        p2 = ExitStack()
        avp = p2.enter_context(tc.tile_pool(name="avp", bufs=2, space="PSUM"))
        expt_pool = p2.enter_context(tc.tile_pool(name="expt", bufs=16))
        nrm_pool = p2.enter_context(tc.tile_pool(name="nrm", bufs=2))

        expt = {}
        av_tiles = {}

        def emit_scores_exp(hg, kc):
            """sc psum [128 ktok, 2 heads x 512 qtok] -> exp w/ rstd_k scale."""
            sc = PA.tile([128, 1024], F32, tag="pa", name=f"sc{hg}_{kc}")
            c4, half = hg // 2, hg % 2
            for hh in range(2):
                nc.tensor.matmul(
                    sc[:, hh * TOK:(hh + 1) * TOK],
                    ktf[hg][hh * 64:hh * 64 + 64, kc * 128:(kc + 1) * 128],
                    qhat[c4][hh * 64:hh * 64 + 64,
                             half * TOK:(half + 1) * TOK],
                    start=True, stop=True,
                )
            e = expt_pool.tile([128, 1024], BF, tag="expt", name="expt")
            nc.scalar.activation(e[:], sc[:], AF.Exp,
                                 scale=rstdk_t[:, kc:kc + 1])
            expt[(hg, kc)] = e

        def emit_av(hg, kc):
            if kc == 0:
                av_tiles[hg] = avp.tile([65, 1024], F32, tag="av", name="av")
            av = av_tiles[hg]
            for hh in range(2):
                h = hg * 2 + hh
                nc.tensor.matmul(
                    av[:, hh * TOK:(hh + 1) * TOK],
                    vaug[kc][:, h * 65:(h + 1) * 65],
                    expt[(hg, kc)][:, hh * TOK:(hh + 1) * TOK],
                    start=(kc == 0), stop=(kc == KC - 1),
                )
            del expt[(hg, kc)]

        def emit_normalize(hg):
            av = av_tiles.pop(hg)
            rec = nrm_pool.tile([1, 1024], BF, tag="rec", name="rec")
            with nc.allow_low_precision(reason="bf16 softmax recip ok"):
                nc.vector.reciprocal(out=rec[:], in_=av[64:65, :])
            rb = nrm_pool.tile([64, 1024], BF, tag="rb", name="rb")
            nc.gpsimd.partition_broadcast(rb[:], rec[:])
            for hh in range(2):
                nc.vector.tensor_mul(
                    attnT[hg][hh * 64:hh * 64 + 64, :],
                    av[0:64, hh * TOK:(hh + 1) * TOK],
                    rb[:, hh * TOK:(hh + 1) * TOK],
                )

        for hg in range(9):
            for kc in range(KC):
                if hg >= 1:
                    emit_av(hg - 1, kc)
                if hg < 8:
                    emit_scores_exp(hg, kc)
            if hg == 5:
                wo_res = wo_pool.tile([128, 8, 1024], BF, tag="wores",
                                      name="wores")
                for ci in range(8):
                    nc.sync.dma_start(
                        out=wo_res[:, ci, :],
                        in_=wout.ap()[ci * 128:(ci + 1) * 128, :],
                    )
            if hg >= 1:
                emit_normalize(hg - 1)
            if dbg == "expt0" and hg == 0:
                dump8([expt[(0, kc)] for kc in range(8)], cols=slice(0, TOK))

        p2.close()

        # ---------------- phase 3: output projection ----------------
        p3 = ExitStack()
        ob_pool = p3.enter_context(tc.tile_pool(name="ob", bufs=2))
        for pair in range(4):
            ps = PA.tile([128, 1024], F32, tag="pa", name=f"outp{pair}")
            for half in range(2):
                co = 2 * pair + half
                for ci in range(8):
                    nc.tensor.matmul(
                        ps[:, half * TOK:(half + 1) * TOK],
                        wo_res[:, ci, co * 128:(co + 1) * 128],
                        attnT[ci][:],
                        start=(ci == 0), stop=(ci == 7),
                    )
            osb = ob_pool.tile([128, 1024], F32, tag="osb", name="osb")
            nc.vector.tensor_copy(osb[:], ps[:])
            oeng = [nc.sync, nc.scalar, nc.gpsimd, nc.sync][pair]
            oeng.dma_start(
                out=bass.AP(tensor=out.ap().tensor,
                            offset=(2 * pair) * 128 * TOK,
                            ap=[[TOK, 128], [128 * TOK, 2], [1, TOK]]),
                in_=osb[:].rearrange("p (h t) -> p h t", h=2),
            )
        p3.close()

    nc.compile()
    return nc


def _host_prep(inputs):
    import ml_dtypes

    bf16 = ml_dtypes.bfloat16
    x = np.asarray(inputs["x"], np.float32)
    Wqkv = np.asarray(inputs["Wqkv"], np.float32)
    bqkv = np.asarray(inputs["bqkv"], np.float32)
    qs = np.asarray(inputs["q_scale"], np.float32)
    ks = np.asarray(inputs["k_scale"], np.float32)
    Wout = np.asarray(inputs["Wout"], np.float32)

    # per-head channel order: evens of pairs 0-15, odds of pairs 0-15,
    # evens of pairs 16-31, odds of pairs 16-31 (so the rope x1<->x2 swap is
    # a +-16-partition shuffle within 32-blocks, expressible on the DVE)
    p64 = np.concatenate([
        np.arange(0, 32, 2), np.arange(1, 32, 2),
        np.arange(32, 64, 2), np.arange(33, 64, 2),
    ])
    perm = np.concatenate([64 * h + p64 for h in range(H)])

    qsp, ksp = qs[perm], ks[perm]
    Wq = Wqkv[:, :DIM][:, perm] * qsp[None, :]
    Wk = Wqkv[:, DIM:2 * DIM][:, perm] * ksp[None, :]
    Wv = Wqkv[:, 2 * DIM:]
    W = np.concatenate([Wq, Wk, Wv], 1).astype(bf16)
    bq = bqkv[:DIM][perm] * qsp
    bk = bqkv[DIM:2 * DIM][perm] * ksp
    bias = np.concatenate([bq, bk, bqkv[2 * DIM:]])[None, :].astype(bf16)
    wsum = np.stack(
        [1.0 / (DIM * qsp ** 2), 1.0 / (DIM * ksp ** 2)], 1
    ).astype(bf16)

    inv_freq = 1.0 / (BASE ** (np.arange(0, HD, 2).astype(np.float32) / HD))
    pos = np.maximum(np.arange(N) - 1, 0).astype(np.float32)
    ang = pos[:, None] * inv_freq[None, :]
    cosT, sinT = np.cos(ang).T, np.sin(ang).T           # (32, N)
    C64 = np.concatenate([cosT[0:16], cosT[0:16], cosT[16:32], cosT[16:32]], 0)
    S64 = np.concatenate([-sinT[0:16], sinT[0:16], -sinT[16:32], sinT[16:32]], 0)
    C128 = np.tile(C64, (2, 1))                          # (128, N)
    S128 = np.tile(S64, (2, 1))

    in_maps = []
    for core in range(NCORE):
        b, sh = core // 4, core % 4
        t0 = sh * TOK
        xTs = np.ascontiguousarray(x[b, t0:t0 + TOK, :].T).astype(bf16)
        tabs = np.stack([
            C128[:, t0:t0 + TOK] * 0.125,
            S128[:, t0:t0 + TOK] * 0.125,
            C128[:, t0:t0 + TOK],
            S128[:, t0:t0 + TOK],
        ]).astype(bf16)
        in_maps.append({
            "xT": xTs,
            "wqkv": W,
            "bqkv": bias,
            "wsum": wsum,
            "tabs": np.ascontiguousarray(tabs),
            "wout": Wout.astype(bf16),
        })
    return in_maps


LAST_EXEC_NS = None


def kernel(**inputs):
    global LAST_EXEC_NS
    import os
    from concourse.bass_utils import run_bass_kernel_spmd

    dbg = os.environ.get("KERNEL_DBG") or None
    zb = bool(np.all(np.asarray(inputs["bqkv"]) == 0))
    key = f"nc{dbg}{zb}"
    if key not in _CACHE:
        _CACHE[key] = _build_nc(dbg, zero_bias=zb)
    nc = _CACHE[key]

    in_maps = _host_prep(inputs)
    trace = bool(int(os.environ.get("KERNEL_TRACE", "0")))
    tmpdir = None
    if trace:
        import tempfile
        import concourse.bass_utils as _bu
        _bu.upload_artifacts = lambda d: d  # keep artifacts local
        tmpdir = tempfile.mkdtemp(prefix="ktrace_")
        print("TRACE DIR:", tmpdir)
    res = run_bass_kernel_spmd(
        nc, in_maps, core_ids=list(range(NCORE)), trace=trace, tmpdir=tmpdir
    )
    LAST_EXEC_NS = res.exec_time_ns
    bout = np.asarray(inputs["bout"], np.float32)
    out = np.empty((B, N, DIN), np.float32)
    for core in range(NCORE):
        b, sh = core // 4, core % 4
        t0 = sh * TOK
        out[b, t0:t0 + TOK, :] = res.results[core]["out"].T
    out += bout[None, None, :]
    return out


def kernel_raw(inputs):
    """Debug helper: run and return the per-core raw [1024, 512] outputs."""
    global LAST_EXEC_NS
    import os
    from concourse.bass_utils import run_bass_kernel_spmd

    dbg = os.environ.get("KERNEL_DBG") or None
    zb = bool(np.all(np.asarray(inputs["bqkv"]) == 0))
    key = f"nc{dbg}{zb}"
    if key not in _CACHE:
        _CACHE[key] = _build_nc(dbg, zero_bias=zb)
    nc = _CACHE[key]
    in_maps = _host_prep(inputs)
    res = run_bass_kernel_spmd(nc, in_maps, core_ids=list(range(NCORE)))
    LAST_EXEC_NS = res.exec_time_ns
    key = "dbg" if dbg else "out"
    return [r[key] for r in res.results]
